# revision 65
# baseline (speedup 1.0000x reference)
"""Trainium2 Bass kernel for the ExpCloudMMD loss.

reference math (gamma = 0.5):
  t1 = mean_{j,k} exp(-g*||p_j - p_k||^2)            over [8192, 8192]
  t2 = 2/(Nx*Np) * sum_{i,j} exp(-g*||x_i - p_j||^2) over [32768, 8192]
  out = t1 - t2  (f32 scalar)

Strategy (8 cores, SPMD, no collectives):
  - t2: shard x rows 8-way; each core computes its 4096x8192 cross block.
  - t1: symmetric; 2048x2048 super-blocks: diagonal (4) + strict upper (6)
    of the 4x4 grid, upper counted twice.  The x2 weight is folded into the
    *rhs encoding* (u2 -= 2*ln2 adds +ln2 to the exp argument), so every
    computed element carries its final weight and all sums are uniform.
    The 160 (row-block, col-group) pairs are dealt round-robin to the 8
    cores; each core's pair list rides in its `pslhs` input tensor, so the
    program stays identical across cores.
  - The exp *argument* p.x - g|x|^2 - g|p|^2 is produced directly by
    K=68 matmuls using an augmented bf16 hi/lo encoding.
  - The per-core workload is one flat stream of 299008 argument columns
    (262144 cross, then 36864 t1).  A deficit scheduler chops it into
    whole rounds owned by one engine each:
      * ScalarE rounds: [128, 1536] PSUM tile (3 banks x 2 bufs),
        activation(Exp, accum_out) in place; ~1.07 ns/col on HW.
      * VectorE rounds: [128, 1024] PSUM tile (2 banks x 1 buf), exp via
        2 single-uop custom DVE ops: pass1 p4 = poly3(x)^4 (deg-3 fit of
        exp(x/512), 8 ALU stages) into an f32 stage tile; pass2
        (p4)^128 = exp(x) with fused accum.  Pass2 lags pass1 by one
        round (the same-round write->read turnaround measurably stalls
        HW), and the P/S alternation covers the single psv buffer's
        refill window.  ~2.8 ns/col on HW (the accum_out adds ~0.45
        ns/element on HW vs TimelineSim — measured, not modeled).
    PE (args) runs at ~52%.
  - Host combine: per-plan-round accumulator columns (acc for ScalarE,
    accd for VectorE) classified cross vs t1 by stream position.
  - Measured: 248.7us vs the 306.3us single-engine baseline (rel err
    2.7e-4); TimelineSim models 240us (the DVE accum tax is not in the
    model).  Failed directions (measured slower): batching pass2 4-8x
    (dvacc2/dvacc3 — per-element accum cost doesn't amortize and
    back-to-back pass1's stall on the 1-buffer refill), SBUF-out
    activation, >512-wide matmuls (backend rejects), splitting one PSUM
    tile between both engines (tensor-granular deps serialize the
    pipeline on the last consumer).
"""

import threading

import ml_dtypes
import numpy as np

import concourse.bass as bass  # noqa: F401
import concourse.mybir as mybir
import concourse.tile as tile
from concourse import bacc, bass_utils

bf16 = ml_dtypes.bfloat16

GAMMA = 0.5
NX, NP, D = 32768, 8192, 16
N_CORES = 8
XS = NX // N_CORES     # 4096 x rows per core
K = 68                 # 4*16 (hi/lo product blocks) + 2 + 2 norm channels

# t1 coarse-triangle schedule: for col-super-group g (2048 particles),
# the computed row-blocks are the 16*(g+1) blocks of super-rows 0..g,
# dealt round-robin (r % 8) to cores -> per-core counts 2,4,6,8.
T1_COUNTS = [2, 4, 6, 8]
N_T1_PAIRS = sum(T1_COUNTS)                    # 20 per core
PS_COLS = N_T1_PAIRS * 128                     # 2560 pslhs columns per core

# v3 stream geometry: the arg stream is chopped into variable-width rounds,
# each assigned wholly to ScalarE (width 1536, PSUM pool of 2) or VectorE
# (width 1024, PSUM pool of 1 — pass2 covers the refill latency), chosen by
# a deficit scheduler over each engine's modeled busy cost.
W_A = 1536                     # ScalarE round width (3 PSUM banks x 2 bufs)
N_STREAM = XS * (NP // 128) + N_T1_PAIRS * 2048 - 4 * 1024  # 299008
CROSS_COLS = XS * (NP // 128)  # 262144; t1 starts here in the stream

DEFAULT_MODE = "dvacc3"

# Per-mode VectorE config: round width, PSUM pool buffers, pass2 batch.
#  - dvacc:  1024-wide, 1 psv buffer, pass2+accum every round (the P/S
#    alternation covers the PSUM refill window).
#  - dvacc3: 512-wide, 2 psv buffers (back-to-back P's, refill prefilled
#    during the previous P), pass2+accum batched over 8 rounds so the
#    ~475ns HW accum_out readout amortizes.
_MODE_CFG = {
    "dvacc": dict(w_v=1024, psv_bufs=1, batch=1, tail="acc"),
    "dvacc2": dict(w_v=1024, psv_bufs=1, batch=4, tail="acc"),
    "dvacc3": dict(w_v=512, psv_bufs=2, batch=8, tail="acc"),
    "noaccum": dict(w_v=1024, psv_bufs=1, batch=1, tail="noacc"),
    "sameop": dict(w_v=1024, psv_bufs=1, batch=1, tail="sameop"),
    # correct-math variant: ScalarE writes bf16 to SBUF instead of in-place
    "dvacc_sbuf": dict(w_v=1024, psv_bufs=1, batch=1, tail="acc",
                       act_sbuf=True),
    # timing diagnostics: every round on ScalarE (no DVE work at all)
    "allA": dict(w_v=1024, psv_bufs=1, batch=1, tail="acc", all_a=True),
    "allA_sbuf": dict(w_v=1024, psv_bufs=1, batch=1, tail="acc", all_a=True,
                      act_sbuf=True),
    "allA_noacc": dict(w_v=1024, psv_bufs=1, batch=1, tail="acc", all_a=True,
                       act_noacc=True),
}


def _cost_a(w):
    return w / 1.2 + 185 + 187


def _cost_v(w, cfg):
    # +100: observed per-round stall from PE in-order queue head-of-line
    # +475: HW cost of the custom-DVE accum_out readout (not in TimelineSim),
    #       amortized over the pass2 batch
    per_round = 100.0 if cfg["batch"] == 1 else 40.0
    return (2 * w / 0.96 + 125 + (60 + 475) / cfg["batch"] + per_round)


_plan_cache = {}


def _round_plan(mode):
    """[(engine, start_col, width)] covering the stream exactly."""
    if mode in _plan_cache:
        return _plan_cache[mode]
    cfg = _MODE_CFG[mode]
    w_v = cfg["w_v"]
    plan = []
    # DVE's first round starts ~3us after ACT's (prologue DMA + PE order)
    ta, tv = 0.0, 3000.0
    pos = 0
    while pos < N_STREAM:
        rem = N_STREAM - pos
        if pos < CROSS_COLS:
            rem = CROSS_COLS - pos  # keep cross / t1 in separate rounds
        if cfg.get("all_a") or \
                ta + _cost_a(min(W_A, rem)) <= tv + _cost_v(min(w_v, rem), cfg):
            w = min(W_A, rem)
            plan.append(("A", pos, w))
            ta += _cost_a(w)
        else:
            w = min(w_v, rem)
            plan.append(("V", pos, w))
            tv += _cost_v(w, cfg)
        pos += w
    _plan_cache[mode] = plan
    return plan

LN2 = float(np.log(2.0))


def _t1_pairs(core):
    """[(row_block, col_group, weight)] for this core, in program order."""
    pairs = []
    for g in range(4):
        rows = [r for r in range(16 * (g + 1)) if r % N_CORES == core]
        assert len(rows) == T1_COUNTS[g]
        for r in rows:
            pairs.append((r, g, 1.0 if r // 16 == g else 2.0))
    return pairs


def _t1_segments():
    """The t1 part of the arg stream: (pslhs_slot, rhs_name, rhs_col, width).
    rhs_name "prhs2" carries the x2 weight folded into its encoding.
    Mirrors the validated t1fine schedule of the v1 kernel: per col-group g,
    the first counts[g]-2 slots (off-diagonal rows, weight 2) get the full
    2048 cols; the two diagonal-row slots get [h0 w1, h1 w2] and [h1 w1]."""
    segs = []
    slot = 0
    for g in range(4):
        for _t in range(T1_COUNTS[g] - 2):
            segs.append((slot, "prhs2", g * 2048, 2048))
            slot += 1
        segs.append((slot, "prhs1", g * 2048, 1024))
        segs.append((slot, "prhs2", g * 2048 + 1024, 1024))
        slot += 1
        segs.append((slot, "prhs1", g * 2048 + 1024, 1024))
        slot += 1
    assert slot == N_T1_PAIRS
    assert sum(w for _, _, _, w in segs) == N_STREAM - CROSS_COLS
    return segs


def _round_pieces(plan):
    """Per plan round, list of matmul pieces (lhs, lhs_col, rhs, rhs_col, dst, w)."""
    # Build the flat stream of segments.
    stream = []  # (lhs_name, lhs_col, rhs_name, rhs_col, width)
    for j in range(NP // 128):
        stream.append(("plhs", j * 128, "xrhs", 0, XS))
    for slot, rhs, rc, w in _t1_segments():
        stream.append(("pslhs", slot * 128, rhs, rc, w))
    # Split segments at round boundaries and the 512 grid (the backend
    # rejects matmuls moving more than 512 columns).
    bounds = [start for _e, start, _w in plan] + [N_STREAM]
    rounds = [[] for _ in range(len(plan))]
    pos = 0
    ri = 0
    for lhs, lc, rhs, rc, width in stream:
        off = 0
        while off < width:
            while pos >= bounds[ri + 1]:
                ri += 1
            d = pos - bounds[ri]
            w = min(512 - d % 512, width - off, bounds[ri + 1] - pos)
            rounds[ri].append((lhs, lc, rhs, rc + off, d, w))
            off += w
            pos += w
    assert pos == N_STREAM
    return rounds


# ---- DVE exp: exp(x) = p(x)^512, p = deg-3 fit of exp(x/512) ----


def _fit_exp_coeffs():
    """p(x) = 1 + c1*x + c2*x^2 + c3*x^3 ~= exp(x/512); returns [c1, c2, c3].
    The constant term is pinned to the DVE's hardware `One`, and the bias of
    the fp32 squaring chain is tuned out on a chi2(32)-like argument mix."""
    M = 512.0
    lo, hi = -110.0 / M, 0.1 / M
    k = np.arange(4000)
    y = (lo + hi) / 2 + (hi - lo) / 2 * np.cos((2 * k + 1) * np.pi / (2 * len(k)))
    V = np.vander(y, 3, increasing=True) * y[:, None]
    w = 1.0 / np.exp(y)
    q = np.linalg.lstsq(V * w[:, None], (np.exp(y) - 1.0) * w, rcond=None)[0]
    ct = q / (M ** (np.arange(3) + 1))

    def emu(x, scale):
        c1, c2, c3 = (ct * scale).astype(np.float32)
        x = x.astype(np.float32)
        p = (((x * c3 + c2) * x + c1) * x + np.float32(1.0)).astype(np.float32)
        s = p
        for _ in range(9):
            s = (s * s).astype(np.float32)
        return s

    rng = np.random.default_rng(1)
    d2 = (rng.standard_normal((400000, 16)) * np.sqrt(2)).astype(np.float32)
    args = -0.5 * (d2 ** 2).sum(1)
    ref = np.exp(args.astype(np.float64))

    def bias(scale):
        return (emu(args, scale).sum(dtype=np.float64) - ref.sum()) / ref.sum()

    g1, g2 = bias(1.0), bias(1.0001)
    lam = -g1 / ((g2 - g1) / 0.0001)
    return (ct * (1.0 + lam)).astype(np.float32)


_EXP_CT = _fit_exp_coeffs()
_dve_exp_ops = None


def _register_dve_exp_ops():
    """Define + register the custom DVE ops (idempotent, in-process).
    Returns (opP, opS_accum, opS_noacc)."""
    global _dve_exp_ops
    if _dve_exp_ops is not None:
        return _dve_exp_ops
    from operator import add as _opadd

    import concourse.dve_ops as dom
    from concourse.dve_spec import (
        C0, C1, C2, One, Spec, Src0, _has_src1, lower as _dve_lower, sq,
    )
    from concourse.dve_uop import DveOpSpec

    def _sq(v, n):
        s = v.astype(np.float32)
        for _ in range(n):
            s = (s * s).astype(np.float32)
        return s

    _s7 = Src0
    for _ in range(7):
        _s7 = sq(_s7)
    specs = [
        # p4 = (((c3*x + c2)*x + c1)*x + 1)^4   (constant term = hw One)
        ("ANT_EXPP512_1", Spec(
            body=sq(sq((((Src0 * C0) + C1) * Src0 + C2) * Src0 + One)),
            reference=lambda in0, in1, c0, c1, c2: _sq(
                ((in0.astype(np.float32) * np.float32(c0) + np.float32(c1))
                 * in0 + np.float32(c2)) * in0 + np.float32(1.0), 2
            ),
        )),
        # s = in^128, fused row-sum accumulator
        ("ANT_EXPS512", Spec(
            body=_s7,
            accum=_opadd,
            accum_init=C0,
            reference=dom._ref_body_sum(lambda in0, in1, c0, c1, c2: _sq(in0, 7)),
        )),
        # s = in^128, plain (for the PE-reduce mode)
        ("ANT_EXPS512_NA", Spec(
            body=_s7,
            reference=lambda in0, in1, c0, c1, c2: _sq(in0, 7),
        )),
    ]

    ops = []
    for name, spec in specs:
        if name in dom._SUB_OPCODE_FOR_NAME:
            ops.append(next(o for o in dom.OPS if o.name == name))
            continue
        row = dom._CUSTOM_DVE_ROW_BASE + len(dom.OPS)
        assert row < 0x20, "custom DVE opcode rows exhausted"
        op = dom.DveOp(name, spec, subdim=False, uops_sha={})
        for ver in ("v3", "v4"):
            u = _dve_lower(spec, ver=ver)
            sha = DveOpSpec(
                name=name, opcode=row, uops=u, rd1_en=_has_src1(spec)
            ).sha(ver)
            op.uops_sha[ver] = sha
        dom.OPS.append(op)
        dom._SUB_OPCODE_FOR_NAME[name] = row
        dom.CUSTOM_DVE_SPECS[name] = spec
        ops.append(op)
    _dve_exp_ops = tuple(ops)
    return _dve_exp_ops


def _build_nc(repeats=1, mode=None):
    """Correct-math modes: "dvacc" / "dvacc2" / "dvacc3" (see _MODE_CFG).
    Timing-only diagnostic modes: "noaccum" (pass2 without accumulator),
    "sameop" (pass2 reuses the pass1 op — no uop-table switch)."""
    if mode in (None, "base"):
        mode = DEFAULT_MODE
    cfg = _MODE_CFG[mode]
    W_V = cfg["w_v"]
    SB_PAIR = cfg["batch"]
    plan = _round_plan(mode)
    n_plan = len(plan)
    nc = bacc.Bacc(
        "TRN2",
        target_bir_lowering=False,
        debug=False,
        enable_asserts=False,
        num_devices=N_CORES,
    )
    dt = mybir.dt
    opP, opS, opSN = _register_dve_exp_ops()
    ct = [float(v) for v in _EXP_CT]

    plhs = nc.dram_tensor("plhs", [K, NP], dt.bfloat16, kind="ExternalInput").ap()
    prhs1 = nc.dram_tensor("prhs1", [K, NP], dt.bfloat16, kind="ExternalInput").ap()
    prhs2 = nc.dram_tensor("prhs2", [K, NP], dt.bfloat16, kind="ExternalInput").ap()
    xrhs = nc.dram_tensor("xrhs", [K, XS], dt.bfloat16, kind="ExternalInput").ap()
    pslhs = nc.dram_tensor("pslhs", [K, PS_COLS], dt.bfloat16, kind="ExternalInput").ap()
    acc_d = nc.dram_tensor("acc", [128, n_plan], dt.float32, kind="ExternalOutput").ap()
    accd_d = nc.dram_tensor("accd", [128, n_plan], dt.float32, kind="ExternalOutput").ap()

    rounds = _round_pieces(plan)

    with tile.TileContext(nc) as tc:
        with (
            tc.tile_pool(name="const", bufs=1) as const,
            tc.tile_pool(name="scrp", bufs=2) as scrp,
            tc.tile_pool(name="psa", bufs=2, space="PSUM") as psa,
            tc.tile_pool(name="psv", bufs=cfg["psv_bufs"], space="PSUM") as psv,
            tc.tile_pool(name="stagep", bufs=2) as stagep,
            tc.tile_pool(name="scr3p", bufs=2) as scr3p,
        ):
            sb = {
                "plhs": const.tile([K, NP], dt.bfloat16, name="sb_plhs"),
                "prhs1": const.tile([K, NP], dt.bfloat16, name="sb_prhs1"),
                "prhs2": const.tile([K, NP], dt.bfloat16, name="sb_prhs2"),
                "xrhs": const.tile([K, XS], dt.bfloat16, name="sb_xrhs"),
                "pslhs": const.tile([K, PS_COLS], dt.bfloat16, name="sb_pslhs"),
            }
            sb_acc = const.tile([128, n_plan], dt.float32)
            sb_accd = const.tile([128, n_plan], dt.float32)
            sb_tiny = const.tile([1, 1], dt.float32)

            # Warm the ACT exp table set (~2.7us) and the DVE custom-op uop
            # tables during the DMA prologue.
            sb_tiny2 = const.tile([1, 1], dt.float32)
            nc.gpsimd.memset(sb_tiny[:], 0.0)
            nc.scalar.activation(
                sb_tiny[:], sb_tiny[:], mybir.ActivationFunctionType.Exp
            )
            nc.vector._custom_dve(
                opP, out=sb_tiny2[:], in0=sb_tiny[:],
                s0=ct[2], s1=ct[1], imm2=ct[0],
            )

            # Input loads, in consumption order: the first rounds need only
            # plhs[:, :128] and the leading xrhs columns.  Later loads go on
            # the Activation engine's DGE queue so the first rounds' waits
            # (which conservatively cover earlier same-queue DMAs) stay small.
            nc.sync.dma_start(sb["plhs"][:, 0:128], plhs[:, 0:128])
            nc.sync.dma_start(sb["xrhs"][:, 0:2560], xrhs[:, 0:2560])
            nc.sync.dma_start(sb["xrhs"][:, 2560:XS], xrhs[:, 2560:XS])
            pchunk = NP // 8
            nc.sync.dma_start(sb["plhs"][:, 128:pchunk], plhs[:, 128:pchunk])
            for i in range(1, 8):
                s = slice(i * pchunk, (i + 1) * pchunk)
                nc.sync.dma_start(sb["plhs"][:, s], plhs[:, s])
            nc.sync.dma_start(sb["pslhs"][:], pslhs[:])
            nc.sync.dma_start(sb["prhs1"][:], prhs1[:])
            nc.sync.dma_start(sb["prhs2"][:], prhs2[:])

            if mode == "pered":
                ones = const.tile([128, 1], dt.bfloat16)
                nc.gpsimd.memset(ones[:], 1.0)

            if repeats == 0:
                nc.gpsimd.memset(sb_acc[:], 0.0)
                nc.gpsimd.memset(sb_accd[:], 0.0)

            # The accd columns of ScalarE rounds (and acc columns of VectorE
            # rounds) are never written; zero both before the pipeline.
            nc.gpsimd.memset(sb_acc[:], 0.0)
            nc.gpsimd.memset(sb_accd[:], 0.0)


            for _rep in range(repeats):
                pend = []   # (round, col_off, width) in the open stage tile
                stage = None
                ready = []  # sealed stage batches awaiting pass2 (1-deep lag)

                def seal():
                    nonlocal pend, stage
                    if pend:
                        ready.append(
                            (pend[0][0], stage, pend[-1][1] + pend[-1][2])
                        )
                        pend = []
                        stage = None

                def emit_pass2():
                    r0, st, ncols = ready.pop(0)
                    scr3 = scr3p.tile([128, SB_PAIR * W_V], dt.bfloat16,
                                      tag="scr3")
                    if cfg["tail"] == "acc":
                        nc.vector._custom_dve(
                            opS,
                            out=scr3[:, 0:ncols],
                            in0=st[:, 0:ncols],
                            s0=0.0, s1=0.0,
                            accum_out=sb_accd[:, r0:r0 + 1],
                        )
                    elif cfg["tail"] == "noacc":
                        nc.vector._custom_dve(
                            opSN, out=scr3[:, 0:ncols], in0=st[:, 0:ncols],
                            s0=0.0, s1=0.0,
                        )
                    elif cfg["tail"] == "sameop":
                        nc.vector._custom_dve(
                            opP, out=scr3[:, 0:ncols], in0=st[:, 0:ncols],
                            s0=ct[2], s1=ct[1], imm2=ct[0],
                        )
                    else:
                        raise ValueError(mode)

                for r, (engine, start, w) in enumerate(plan):
                    if start >= CROSS_COLS and pend and \
                            plan[pend[0][0]][1] < CROSS_COLS:
                        seal()  # never mix cross and t1 in one accum batch
                    if engine == "A":
                        ps = psa.tile([128, W_A], dt.float32, tag="psa")
                    else:
                        ps = psv.tile([128, W_V], dt.float32, tag="psv")
                    for lhs, lc, rhs, rc, d, pw in rounds[r]:
                        nc.tensor.matmul(
                            ps[:, d:d + pw],
                            sb[lhs][:, lc:lc + 128],
                            sb[rhs][:, rc:rc + pw],
                        )
                    if engine == "A":
                        if cfg.get("act_sbuf"):
                            scr = scrp.tile([128, W_A], dt.bfloat16, tag="scr")
                            aout = scr[:, 0:w]
                        else:
                            # in-place: PSUM write latency beats SBUF
                            aout = ps[:, 0:w]
                        nc.scalar.activation(
                            aout,
                            ps[:, 0:w],
                            mybir.ActivationFunctionType.Exp,
                            accum_out=(None if cfg.get("act_noacc")
                                       else sb_acc[:, r:r + 1]),
                        )
                    else:
                        if stage is None:
                            stage = stagep.tile([128, SB_PAIR * W_V],
                                                dt.float32, tag="stage")
                            off = 0
                        else:
                            off = pend[-1][1] + pend[-1][2]
                        nc.vector._custom_dve(
                            opP,
                            out=stage[:, off:off + w],
                            in0=ps[:, 0:w],
                            s0=ct[2], s1=ct[1], imm2=ct[0],
                        )
                        pend.append((r, off, w))
                        if len(pend) >= SB_PAIR:
                            seal()
                        # pass2 lags one batch behind pass1, so it never
                        # reads a stage buffer the instant it was written
                        while len(ready) > 1:
                            emit_pass2()
                seal()
                while ready:
                    emit_pass2()

            # Ship the accumulators; split so most columns overlap the tail.
            half = n_plan // 2
            nc.sync.dma_start(acc_d[:, :half], sb_acc[:, :half])
            nc.sync.dma_start(accd_d[:, :half], sb_accd[:, :half])
            nc.sync.dma_start(acc_d[:, half:], sb_acc[:, half:])
            nc.sync.dma_start(accd_d[:, half:], sb_accd[:, half:])

    nc.compile()
    return nc


def _split_hi_lo(v):
    vh = v.astype(bf16)
    vl = (v - vh.astype(np.float32)).astype(bf16)
    return vh, vl


def _enc_lhsT(p):
    """p: [n, 16] f32 -> [K, n] bf16 stationary-side encoding."""
    n = p.shape[0]
    ph, pl = _split_hi_lo(np.ascontiguousarray(p, np.float32))
    p2 = (-GAMMA * (p.astype(np.float64) ** 2).sum(-1)).astype(np.float32)
    p2h, p2l = _split_hi_lo(p2)
    out = np.empty((K, n), bf16)
    out[0:16] = ph.T
    out[16:32] = pl.T
    out[32:48] = ph.T
    out[48:64] = pl.T
    out[64] = p2h
    out[65] = p2l
    out[66] = bf16(-GAMMA)
    out[67] = bf16(-GAMMA)
    return out


def _enc_rhs(u, arg_offset=0.0):
    """u: [n, 16] f32 -> [K, n] bf16 moving-side encoding.
    arg_offset is added to the exp argument (weight folding: ln(w))."""
    n = u.shape[0]
    uh, ul = _split_hi_lo(np.ascontiguousarray(u, np.float32))
    u2 = ((u.astype(np.float64) ** 2).sum(-1)
          - arg_offset / GAMMA).astype(np.float32)
    u2h, u2l = _split_hi_lo(u2)
    out = np.empty((K, n), bf16)
    out[0:16] = uh.T
    out[16:32] = uh.T
    out[32:48] = ul.T
    out[48:64] = ul.T
    out[64] = bf16(1.0)
    out[65] = bf16(1.0)
    out[66] = u2h
    out[67] = u2l
    return out


def _make_in_maps(x, particles):
    plhs = _enc_lhsT(particles)
    prhs1 = _enc_rhs(particles)
    prhs2 = _enc_rhs(particles, arg_offset=LN2)
    in_maps = []
    for c in range(N_CORES):
        pairs = _t1_pairs(c)
        pslhs = np.concatenate(
            [plhs[:, r * 128:(r + 1) * 128] for r, _, _ in pairs], axis=1
        )
        in_maps.append(
            {
                "plhs": plhs,
                "prhs1": prhs1,
                "prhs2": prhs2,
                "xrhs": _enc_rhs(x[c * XS:(c + 1) * XS]),
                "pslhs": np.ascontiguousarray(pslhs),
            }
        )
    return in_maps


def _combine(results):
    plan = _round_plan(DEFAULT_MODE)
    is_cross = np.array([start < CROSS_COLS for _e, start, _w in plan])
    t2_sum = 0.0
    t1_sum = 0.0
    for r in results:
        tot = r["acc"].astype(np.float64) + r["accd"].astype(np.float64)
        t2_sum += tot[:, is_cross].sum()
        t1_sum += tot[:, ~is_cross].sum()
    t1 = t1_sum / (float(NP) * NP)
    t2 = 2.0 * t2_sum / (float(NX) * NP)
    return np.float32(t1 - t2)


_lock = threading.Lock()
_cached_nc = None


def _get_nc():
    global _cached_nc
    with _lock:
        if _cached_nc is None:
            _cached_nc = _build_nc()
        return _cached_nc


def kernel(x, particles):
    x = np.asarray(x, np.float32)
    particles = np.asarray(particles, np.float32)
    assert x.shape == (NX, D) and particles.shape == (NP, D)

    nc = _get_nc()
    in_maps = _make_in_maps(x, particles)
    res = bass_utils.run_bass_kernel_spmd(nc, in_maps, core_ids=list(range(N_CORES)))
    return _combine(res.results)
